# revision 9
# baseline (speedup 1.0000x reference)
"""Causal self-attention (B=4, T=2048, C=1024, H=16) on 8 trn2 NeuronCores.

Sharding: tensor-parallel over heads. Core c owns heads (2c, 2c+1).
Each core computes QKV projection for its 2 heads (full x), causal
attention for its (4 batches x 2 heads), and a partial output projection
with its 128 rows of W_proj. Host sums the 8 partial outputs + b_proj.

Device-side layout choices:
  - x is transposed on load (DMA xbar transpose, bf16) to x_T [c_in, t]
  - Q,K are produced transposed: [c_out(=2*64), t] so the scores matmul
    contracts d on partitions; heads live on partition halves 0:64/64:128
    which row-packs the two heads' score matmuls in the PE array.
  - scores S_T [k, q] per 128-k-block; exp on ACT (scale=1/8 folded in),
    causal handled by block skipping + a static triangular 0/1 mask.
  - softmax sums via N=1 matmuls (exp_block.T @ ones) accumulated as
    columns of a per-batch PSUM tile -> reciprocal in column form.
  - AV matmul accumulates y_T [d(2 heads on partition halves), q].
  - normalization: r transposed to rows (PE transpose), broadcast over
    partitions with a tiny selector matmul, multiplied into y on DVE.
  - projection: y_T tiles are lhsT directly; partial out DMAd from PSUM.
"""

import sys

sys.path.insert(0, "/opt/trn_rl_repo")

import numpy as np
import ml_dtypes

B, T, C, H = 4, 2048, 1024, 16
HD = C // H  # 64
BT = B * T  # 8192
NCORES = 8
TCH = 512  # t-chunk
NT = BT // TCH  # 16
NCC = C // 128  # 8 c_in chunks
KB = 128  # k block
QB = 128  # q subblock

_RUNNER = None


def _build_nc():
    import concourse.bacc as bacc
    import concourse.mybir as mybir
    import concourse.tile as tile
    from concourse.masks import make_identity

    f32 = mybir.dt.float32
    bf16 = mybir.dt.bfloat16
    Exp = mybir.ActivationFunctionType.Exp

    nc = bacc.Bacc(None, target_bir_lowering=False, debug=False)

    xbf = nc.dram_tensor("xbf", [BT, C], bf16, kind="ExternalInput")
    wqkv = nc.dram_tensor("wqkv", [C, 384], bf16, kind="ExternalInput")
    bqkv = nc.dram_tensor("bqkv", [384], f32, kind="ExternalInput")
    wproj = nc.dram_tensor("wproj", [128, C], bf16, kind="ExternalInput")
    trimask = nc.dram_tensor("trimask", [128, 128], bf16, kind="ExternalInput")
    sel2 = nc.dram_tensor("sel2", [2, 128], f32, kind="ExternalInput")
    out_d = nc.dram_tensor("out", [BT, C], f32, kind="ExternalOutput")

    with tile.TileContext(nc) as tc:
        with (
            tc.tile_pool(name="const", bufs=1) as const_pool,
            tc.tile_pool(name="big", bufs=1) as big_pool,
            tc.tile_pool(name="sb", bufs=2) as sb_pool,
            tc.tile_pool(name="ps", bufs=1, space="PSUM") as ps_pool,
        ):
            # --- constants ---
            wqkv_sb = const_pool.tile([128, NCC, 384], bf16)
            nc.sync.dma_start(
                wqkv_sb, wqkv.ap().rearrange("(n p) m -> p n m", p=128)
            )
            wproj_sb = const_pool.tile([128, C], bf16)
            nc.sync.dma_start(wproj_sb, wproj.ap())
            bias_sb = const_pool.tile([128, 3], f32)
            nc.sync.dma_start(
                bias_sb, bqkv.ap().rearrange("(n p) -> p n", p=128)
            )
            tri_sb = const_pool.tile([128, 128], bf16)
            nc.sync.dma_start(tri_sb, trimask.ap())
            sel2_sb = const_pool.tile([2, 128], f32)
            nc.sync.dma_start(sel2_sb, sel2.ap())
            ones_sb = const_pool.tile([128, 1], bf16)
            nc.vector.memset(ones_sb, 1.0)
            ident_sb = const_pool.tile([128, 128], f32)
            make_identity(nc, ident_sb)

            # --- persistent activations ---
            qt_sb = big_pool.tile([128, BT], bf16)  # Q_T [2*64, t]
            kt_sb = big_pool.tile([128, BT], bf16)  # K_T
            v_sb = big_pool.tile([128, BT // 128, 128], bf16)  # V [t, c]
            yt_sb = big_pool.tile([128, BT], bf16)  # y_T [c, t]

            # ---------------- phase 1: QKV projection ----------------
            for tch in range(NT):
                t0 = tch * TCH
                xt = sb_pool.tile([128, NCC, TCH], bf16, tag="xt")
                for cc in range(NCC):
                    nc.sync.dma_start_transpose(
                        xt[:, cc, :],
                        xbf.ap()[t0 : t0 + TCH, cc * 128 : (cc + 1) * 128],
                    )
                for o3 in range(3):  # Q_T, K_T, V_T
                    ps = ps_pool.tile([128, TCH], f32, tag="bank", bufs=6)
                    for cc in range(NCC):
                        nc.tensor.matmul(
                            ps,
                            lhsT=wqkv_sb[:, cc, o3 * 128 : (o3 + 1) * 128],
                            rhs=xt[:, cc, :],
                            start=(cc == 0),
                            stop=(cc == NCC - 1),
                        )
                    if o3 == 0:
                        nc.vector.tensor_scalar_add(
                            qt_sb[:, t0 : t0 + TCH], ps, bias_sb[:, 0:1]
                        )
                    elif o3 == 1:
                        nc.vector.tensor_scalar_add(
                            kt_sb[:, t0 : t0 + TCH], ps, bias_sb[:, 1:2]
                        )
                    else:
                        vtmp = sb_pool.tile([128, TCH], bf16, tag="vtmp")
                        nc.vector.tensor_scalar_add(vtmp, ps, bias_sb[:, 2:3])
                        for j in range(TCH // 128):
                            nc.sync.dma_start_transpose(
                                v_sb[:, tch * 4 + j, :],
                                vtmp[:, j * 128 : (j + 1) * 128],
                            )

            # ---------------- phase 2: attention ----------------
            for b in range(B):
                scol_ps = ps_pool.tile([128, 32], f32, tag="scol", bufs=1)
                rcol_sb = sb_pool.tile([128, 32], f32, tag="rcol")
                s_first = [True]

                for qc in range(4):
                    q0 = (b * 4 + qc) * TCH
                    nkb = 4 * qc + 4
                    exps = [
                        sb_pool.tile(
                            [128, 16, TCH],
                            bf16,
                            tag=f"exps{h}",
                            bufs=1,
                            name=f"exps{h}_{b}_{qc}",
                        )
                        for h in (0, 1)
                    ]
                    y2 = ps_pool.tile([128, TCH], f32, tag="bank", bufs=6)
                    for h in (0, 1):
                        hp = h * 64
                        for kb in range(nkb):
                            k0 = (b * 16 + kb) * 128
                            j = kb - 4 * qc
                            c0 = max(0, 128 * j)
                            sps = ps_pool.tile([128, TCH], f32, tag="bank", bufs=6)
                            nc.tensor.matmul(
                                sps,
                                lhsT=kt_sb[hp : hp + 64, k0 : k0 + 128],
                                rhs=qt_sb[hp : hp + 64, q0 : q0 + TCH],
                                start=True,
                                stop=True,
                            )
                            nc.scalar.activation(
                                exps[h][:, kb, c0:TCH],
                                sps[:, c0:TCH],
                                Exp,
                                scale=0.125,
                            )
                            if j >= 0:
                                nc.vector.tensor_mul(
                                    exps[h][:, kb, c0 : c0 + 128],
                                    exps[h][:, kb, c0 : c0 + 128],
                                    tri_sb,
                                )
                        # AV accumulate
                        for kb in range(nkb):
                            j = kb - 4 * qc
                            c0 = max(0, 128 * j)
                            nc.tensor.matmul(
                                y2[hp : hp + 64, c0:TCH],
                                lhsT=v_sb[:, b * 16 + kb, hp : hp + 64],
                                rhs=exps[h][:, kb, c0:TCH],
                                start=(kb == 0),
                                stop=(kb == nkb - 1),
                                tile_position=(0, hp),
                            )
                        # softmax-sum columns: one long accumulation group
                        for jj in range(4):
                            j2 = 4 * qc + jj
                            for kb in range(j2 + 1):
                                is_last = (
                                    qc == 3 and h == 1 and jj == 3 and kb == j2
                                )
                                nc.tensor.matmul(
                                    scol_ps[:, h * 16 + j2 : h * 16 + j2 + 1],
                                    lhsT=exps[h][:, kb, jj * 128 : (jj + 1) * 128],
                                    rhs=ones_sb,
                                    start=s_first[0],
                                    stop=is_last,
                                )
                                s_first[0] = False

                    # normalization for this q-chunk
                    nc.vector.reciprocal(
                        rcol_sb[:, 4 * qc : 4 * qc + 4],
                        scol_ps[:, 4 * qc : 4 * qc + 4],
                    )
                    nc.vector.reciprocal(
                        rcol_sb[:, 16 + 4 * qc : 16 + 4 * qc + 4],
                        scol_ps[:, 16 + 4 * qc : 16 + 4 * qc + 4],
                    )
                    # transpose r columns to rows: per jj a [128,2] slice
                    # (h0,h1 cols) -> [2,128] rows, all at base partition 0
                    rview = rcol_sb.rearrange("p (h j) -> p j h", h=2)[
                        :, 4 * qc : 4 * qc + 4, :
                    ]
                    rrows_ps = ps_pool.tile([2, TCH], f32, tag="rrows", bufs=1)
                    for jj in range(4):
                        nc.tensor.transpose(
                            rrows_ps[:, jj * 128 : (jj + 1) * 128],
                            rview[:, jj, :],
                            ident_sb,
                        )
                    rrows_sb = sb_pool.tile([2, TCH], f32, tag="rrows_sb")
                    nc.vector.tensor_copy(rrows_sb, rrows_ps)
                    rb_ps = ps_pool.tile([128, TCH], f32, tag="bank", bufs=6)
                    for jj in range(4):
                        nc.tensor.matmul(
                            rb_ps[:, jj * 128 : (jj + 1) * 128],
                            lhsT=sel2_sb,
                            rhs=rrows_sb[:, jj * 128 : (jj + 1) * 128],
                            start=True,
                            stop=True,
                        )
                    rb_sb = sb_pool.tile([128, TCH], f32, tag="rb")
                    nc.vector.tensor_copy(rb_sb, rb_ps)
                    nc.vector.tensor_mul(yt_sb[:, q0 : q0 + TCH], y2, rb_sb)

                # ---------------- phase 3: projection (partial) ----------
                for tb in range(16):
                    tg = b * 16 + tb
                    for half in range(2):
                        pps = ps_pool.tile([128, TCH], f32, tag="bank", bufs=6)
                        nc.tensor.matmul(
                            pps,
                            lhsT=yt_sb[:, tg * 128 : (tg + 1) * 128],
                            rhs=wproj_sb[:, half * TCH : (half + 1) * TCH],
                            start=True,
                            stop=True,
                        )
                        oevac = sb_pool.tile([128, TCH], f32, tag="oevac", bufs=3)
                        nc.vector.tensor_copy(oevac, pps)
                        nc.sync.dma_start(
                            out_d.ap()[
                                tg * 128 : (tg + 1) * 128,
                                half * TCH : (half + 1) * TCH,
                            ],
                            oevac,
                        )

    nc.compile()
    return nc


class Runner:
    """Builds the Bass program once and keeps a reusable jitted executor."""

    def __init__(self):
        self.nc = _build_nc()
        self._jit = None
        self._meta = None

    def _build_jit(self):
        import jax
        import numpy as np
        from jax.sharding import Mesh, PartitionSpec
        from jax.experimental.shard_map import shard_map
        import concourse.mybir as mybir
        from concourse import bass2jax

        nc = self.nc
        bass2jax.install_neuronx_cc_hook()

        partition_name = (
            nc.partition_id_tensor.name if nc.partition_id_tensor else None
        )
        in_names, out_names, out_avals = [], [], []
        for alloc in nc.m.functions[0].allocations:
            if not isinstance(alloc, mybir.MemoryLocationSet):
                continue
            name = alloc.memorylocations[0].name
            if alloc.kind == "ExternalInput":
                if name != partition_name:
                    in_names.append(name)
            elif alloc.kind == "ExternalOutput":
                out_names.append(name)
                out_avals.append(
                    jax.core.ShapedArray(
                        tuple(alloc.tensor_shape), mybir.dt.np(alloc.dtype)
                    )
                )
        n_params = len(in_names)
        n_outs = len(out_avals)
        all_in = list(in_names) + list(out_names)
        if partition_name is not None:
            all_in.append(partition_name)

        def _body(*args):
            operands = list(args)
            if partition_name is not None:
                operands.append(bass2jax.partition_id_tensor())
            outs = bass2jax._bass_exec_p.bind(
                *operands,
                out_avals=tuple(out_avals),
                in_names=tuple(all_in),
                out_names=tuple(out_names),
                lowering_input_output_aliases=(),
                sim_require_finite=True,
                sim_require_nnan=True,
                nc=nc,
            )
            return tuple(outs)

        devices = jax.devices()[:NCORES]
        mesh = Mesh(np.asarray(devices), ("core",))
        donate = tuple(range(n_params, n_params + n_outs))
        sharded = jax.jit(
            shard_map(
                _body,
                mesh=mesh,
                in_specs=(PartitionSpec("core"),) * (n_params + n_outs),
                out_specs=(PartitionSpec("core"),) * n_outs,
                check_rep=False,
            ),
            donate_argnums=donate,
            keep_unused=True,
        )
        self._jit = sharded
        self._meta = (in_names, out_names, out_avals)

    def build_timer(self, in_maps, iters):
        """Returns a zero-transfer callable running `iters` chained kernel
        executions on device; inputs are staged on device once."""
        import jax
        import jax.numpy as jnp
        import numpy as np
        from jax.sharding import Mesh, PartitionSpec, NamedSharding
        from jax.experimental.shard_map import shard_map
        import concourse.mybir as mybir
        from concourse import bass2jax

        if self._jit is None:
            self._build_jit()
        nc = self.nc
        in_names, out_names, out_avals = self._meta
        partition_name = (
            nc.partition_id_tensor.name if nc.partition_id_tensor else None
        )
        all_in = list(in_names) + list(out_names)
        if partition_name is not None:
            all_in.append(partition_name)

        n_params = len(in_names)

        def _body(*args):
            ins = list(args[:n_params])
            zeros = list(args[n_params:])
            outs = None
            for _ in range(iters):
                operands = list(ins) + list(zeros)
                if partition_name is not None:
                    operands.append(bass2jax.partition_id_tensor())
                outs = bass2jax._bass_exec_p.bind(
                    *operands,
                    out_avals=tuple(out_avals),
                    in_names=tuple(all_in),
                    out_names=tuple(out_names),
                    lowering_input_output_aliases=(),
                    sim_require_finite=True,
                    sim_require_nnan=True,
                    nc=nc,
                )
            return tuple(outs)

        devices = jax.devices()[:NCORES]
        mesh = Mesh(np.asarray(devices), ("core",))
        spec = NamedSharding(mesh, PartitionSpec("core"))
        fn = jax.jit(
            shard_map(
                _body,
                mesh=mesh,
                in_specs=(PartitionSpec("core"),)
                * (len(in_names) + len(out_names)),
                out_specs=(PartitionSpec("core"),) * len(out_names),
                check_rep=False,
            ),
            keep_unused=True,
        )
        concat_in = [
            jax.device_put(
                np.concatenate([np.asarray(m[name]) for m in in_maps], axis=0),
                spec,
            )
            for name in in_names
        ]
        concat_in += [
            jax.device_put(
                np.zeros((NCORES * a.shape[0], *a.shape[1:]), a.dtype), spec
            )
            for a in out_avals
        ]
        for a in concat_in:
            a.block_until_ready()

        def run():
            outs = fn(*concat_in)
            jax.block_until_ready(outs)
            return outs

        return run

    def execute(self, in_maps):
        """in_maps: list of 8 dicts name->np array. Returns list of out dicts."""
        import numpy as np

        if self._jit is None:
            self._build_jit()
        in_names, out_names, out_avals = self._meta
        concat_in = [
            np.concatenate([np.asarray(m[name]) for m in in_maps], axis=0)
            for name in in_names
        ]
        concat_zeros = [
            np.zeros((NCORES * a.shape[0], *a.shape[1:]), a.dtype)
            for a in out_avals
        ]
        out_arrs = self._jit(*concat_in, *concat_zeros)
        return [
            {
                name: np.asarray(out_arrs[i]).reshape(
                    NCORES, *out_avals[i].shape
                )[c]
                for i, name in enumerate(out_names)
            }
            for c in range(NCORES)
        ]


def make_in_maps(x, W_attn, b_attn, W_proj, b_proj):
    bf16 = ml_dtypes.bfloat16
    xbf = np.ascontiguousarray(x.reshape(BT, C)).astype(bf16)
    tri = np.tril(np.ones((128, 128), np.float32)).T.astype(bf16)
    # trimask[p, c] = 1 if p <= c  (k index on partitions, q on cols)
    sel2 = np.zeros((2, 128), np.float32)
    sel2[0, :64] = 1.0
    sel2[1, 64:] = 1.0
    in_maps = []
    for c in range(NCORES):
        h0 = 2 * c
        cols = np.r_[h0 * HD : (h0 + 2) * HD]
        wq = W_attn[:, cols]
        wk = W_attn[:, C + cols]
        wv = W_attn[:, 2 * C + cols]
        wqkv = np.concatenate([wq, wk, wv], axis=1).astype(bf16)
        bqkv = np.concatenate(
            [b_attn[cols], b_attn[C + cols], b_attn[2 * C + cols]]
        ).astype(np.float32)
        wproj = np.ascontiguousarray(W_proj[cols, :]).astype(bf16)
        in_maps.append(
            {
                "xbf": xbf,
                "wqkv": np.ascontiguousarray(wqkv),
                "bqkv": bqkv,
                "wproj": wproj,
                "trimask": np.ascontiguousarray(tri),
                "sel2": sel2,
            }
        )
    return in_maps


def get_runner():
    global _RUNNER
    if _RUNNER is None:
        _RUNNER = Runner()
    return _RUNNER


def kernel(x, W_attn, b_attn, W_proj, b_proj):
    x = np.asarray(x, dtype=np.float32)
    W_attn = np.asarray(W_attn, dtype=np.float32)
    b_attn = np.asarray(b_attn, dtype=np.float32)
    W_proj = np.asarray(W_proj, dtype=np.float32)
    b_proj = np.asarray(b_proj, dtype=np.float32)
    runner = get_runner()
    in_maps = make_in_maps(x, W_attn, b_attn, W_proj, b_proj)
    results = runner.execute(in_maps)
    total = np.zeros((BT, C), np.float32)
    for r in results:
        total += r["out"]
    total += b_proj[None, :]
    return total.reshape(B, T, C)
